# revision 23
# baseline (speedup 1.0000x reference)
"""GwcVolume (group-wise correlation volume) Bass kernel for Trainium2.

Problem: left/right features [2, 320, 96, 312] fp32, GROUP=40, cpg=8,
max_disp=48.  Output cost volume [2, 40, 48, 96, 312]:
    cost[b,g,d,h,w] = mean_c( l[b,g,c,h,w] * r[b,g,c,h,w-d] ),  0 for w<d.

Strategy (8 NeuronCores):
  - Shard the 80 (b,g) pairs across cores, 10 per core.  Each pair is fully
    independent (no collectives).
  - TensorE does all multiply-accumulate work as block-diagonal matmuls:
    for each (bg, h-group of 16), SBUF holds l as [128 = 16h x 8c, W] and a
    host-prebuilt block-diagonal stationary image rs [128, 10*128] where
    the (unit, w'-block blk, h-quad q) stationary is
        rs[32q + 8hi + c, 128 blk + 32 hi + ww] = r[h, c, 32 blk + ww] / 0,
    h = 16 hg + 4 q + hi.  matmul (K=32 rows at strip 32q, M=128, N=79):
        out[(hi,ww), n] = sum_c r[h,c,w'0+ww] * l[h,c,w'0+n]
                        = cost[d=n-ww, h, w=w'0+n]  for 0 <= n-ww < 48.
    The 4 quads run on distinct PE row-strips and distinct PSUM banks,
    so they execute concurrently on the 32x32 sub-array grid.
  - VectorE/ScalarE evacuate PSUM into a w-major SBUF buffer, DMA'd to HBM
    densely.  The host does the final (free) rearrangement: band extraction
    (d = n - ww), zero triangle for w < d, and the layout transpose.

The device never performs the (d,w)-diagonal transpose -- that keeps every
DMA fully contiguous; the host does it with numpy stride tricks.
"""

import os

import numpy as np

# --- geometry (hardcoded for this problem) ---
B, G, CPG, H, W = 2, 40, 8, 96, 312
D = 48                      # max_disp
N_CORES = 8
PAIRS = B * G               # 80 (b,g) pairs
BG_PER_CORE = PAIRS // N_CORES  # 10
HGROUPS = H // 16           # 6 groups of 16 h's
NBLK = 10                   # w'-blocks of 32 (covers w' in [0, 320))
MBLK = 32                   # w' per block
NW = MBLK + D - 1           # 79 moving columns per matmul
WL = 368                    # padded l width (312 + 56; max needed w = 366)
WR = 320                    # padded r width (312 + 8)
UNITS = BG_PER_CORE * HGROUPS   # 60 (bg, hgroup) units per core
RSW = NBLK * 128            # 1280 stationary-image cols per unit
GU = 6                      # units per DMA group (batch DMAs for efficiency)
NGROUPS = UNITS // GU       # 10

_NC_CACHE = {}


def _build_nc(dt_in_name="bfloat16", dt_out_name="bfloat16"):
    from concourse import bacc, mybir, tile
    import concourse.bass as bass  # noqa: F401

    dt_in = getattr(mybir.dt, dt_in_name)
    dt_out = getattr(mybir.dt, dt_out_name)
    f32 = mybir.dt.float32

    nc = bacc.Bacc("TRN2", target_bir_lowering=False, debug=False)
    l_dram = nc.dram_tensor(
        "l", [NGROUPS, 128, GU, WL], dt_in, kind="ExternalInput")
    r_dram = nc.dram_tensor(
        "rs", [NGROUPS, 128, GU, RSW], dt_in, kind="ExternalInput")
    # Band compaction via slot-major ev layout: stationary M-cols are
    # permuted (host-side) so PSUM partition p = 64k + 16hi + i holds
    # ww = 16k + i, making the valid 48-wide band of partitions [0,64)
    # sit inside moving cols [0,64) and of [64,128) inside [16,80).
    # ev stores the matmul col index ("slot") OUTER-most per partition,
    # so each half's 64-slot window is one contiguous run and the
    # out-DMA extracts a 64/48 = 1.33x band (vs 79/48 = 1.65x) with
    # ~5KB descriptors.  Copies and matmuls are unchanged in count.
    NWM = 80                    # matmul cols (covers window [16,80))
    o_dram = nc.dram_tensor(
        "o", [NGROUPS, 2, 2, 32, GU, 64, 4, NBLK], dt_out,
        kind="ExternalOutput")

    HB = NBLK // 2              # 5 blks per psum tile (one bank per quad)
    SEG = 96                    # col stride of a blk inside its bank
    GH = GU // 2                # out-DMA half-group granularity

    with tile.TileContext(nc) as tc:
        with (
            tc.tile_pool(name="lp", bufs=2) as lp,
            tc.tile_pool(name="rp", bufs=2) as rp,
            tc.tile_pool(name="evp", bufs=2) as evp,
            tc.tile_pool(name="psp", bufs=2, space="PSUM") as psp,
        ):
            for g in range(NGROUPS):
                lt = lp.tile([128, GU, WL], dt_in)
                rt = rp.tile([128, GU, RSW], dt_in)
                # split loads in halves: the first half-group's compute can
                # start while the second half streams in
                for u0 in (0, GH):
                    nc.sync.dma_start(
                        lt[:, u0:u0 + GH, :], l_dram[g, :, u0:u0 + GH, :])
                    nc.sync.dma_start(
                        rt[:, u0:u0 + GH, :], r_dram[g, :, u0:u0 + GH, :])
                ev = evp.tile([128, GU, NWM, 4, NBLK], dt_out)
                for ui in range(GU):
                    for h in range(2):
                        # 5 blks share one psum tile: quad q in bank q,
                        # blk s at col 96*s
                        ps = psp.tile([128, 4, 512], f32)
                        for s in range(HB):
                            blk = h * HB + s
                            for q in range(4):
                                nc.tensor.matmul(
                                    out=ps[:, q, SEG * s:SEG * s + NWM],
                                    lhsT=rt[32 * q:32 * q + 32, ui,
                                            128 * blk:128 * blk + 128],
                                    rhs=lt[32 * q:32 * q + 32, ui,
                                           MBLK * blk:MBLK * blk + NWM],
                                    start=True,
                                    stop=True,
                                    tile_position=(32 * q, 0),
                                )
                        # [p, q, s, n] -> [p, n, q, s] slot-major store
                        src = ps[:, :, 0:HB * SEG].rearrange(
                            "p q (s n) -> p n q s", n=SEG)[:, 0:NWM, :, :]
                        dst = ev[:, ui, :, :, h * HB:(h + 1) * HB]
                        if h == 0:
                            nc.vector.tensor_copy(out=dst, in_=src)
                        else:
                            nc.scalar.copy(out=dst, in_=src)
                    # out-DMAs on the ACT HWDGE ring; each k-group's band
                    # window is read as two 32-partition DMAs of opposite
                    # SDMA-engine parity (p<64 / p>=64), issued adjacently
                    # so all 16 engines run concurrently
                    if ui == GH - 1 or ui == GU - 1:
                        u0 = ui + 1 - GH
                        for k in range(2):
                            for hf in range(2):
                                p0 = 64 * hf + 32 * k
                                nc.scalar.dma_start(
                                    o_dram[g, k, hf, :, u0:u0 + GH],
                                    ev[p0:p0 + 32, u0:u0 + GH,
                                       16 * k:16 * k + 64, :, :])
    nc.compile()
    return nc


def _get_nc(key=("bfloat16", "bfloat16")):
    if key not in _NC_CACHE:
        _NC_CACHE[key] = _build_nc(*key)
    return _NC_CACHE[key]


def _pack_inputs(left, right, dt_np):
    """-> per-core in_maps; l pre-scaled by 1/cpg, r as block-diag image."""
    # [B, C, H, W] -> [B, G, cpg, H, W] -> [pair, H, cpg, W]
    l5 = left.reshape(B, G, CPG, H, W).transpose(0, 1, 3, 2, 4).reshape(
        PAIRS, H, CPG, W)
    r5 = right.reshape(B, G, CPG, H, W).transpose(0, 1, 3, 2, 4).reshape(
        PAIRS, H, CPG, W)
    lp = np.zeros((PAIRS, H, CPG, WL), dtype=np.float32)
    lp[..., :W] = l5 * (1.0 / CPG)
    lp = lp.astype(dt_np)
    # l: [pair, H=6*16, cpg, WL] -> per core [NGROUPS, 128, GU, WL]
    lp = lp.reshape(N_CORES, NGROUPS, GU, 128, WL).transpose(0, 1, 3, 2, 4)

    rp = np.zeros((PAIRS, H, CPG, WR), dtype=np.float32)
    rp[..., :W] = r5
    rp = rp.astype(dt_np)
    # block-diagonal stationary image:
    # axes: [pair, hg, q, hi_row, c, blk, hi_col, ww]
    rv = rp.reshape(PAIRS, HGROUPS, 4, 4, CPG, NBLK, MBLK)
    rb = np.zeros((PAIRS, HGROUPS, 4, 4, CPG, NBLK, 4, MBLK), dtype=dt_np)
    for i in range(4):
        rb[:, :, :, i, :, :, i, :] = rv[:, :, :, i, :, :, :]
    # permute M-cols (hi, ww=16k+8*i3+il) -> (i3, k, hi, il) so PSUM
    # partition p = 64*i3 + 32*k + 8*hi + il: each band-compaction
    # k-group becomes two 32-partition ranges on OPPOSITE sides of
    # p=64 (= opposite SDMA engine parities)
    rb = rb.reshape(PAIRS, HGROUPS, 4, 4, CPG, NBLK, 4, 2, 2, 8)
    rb = np.ascontiguousarray(rb.transpose(0, 1, 2, 3, 4, 5, 8, 7, 6, 9))
    rb = rb.reshape(N_CORES, NGROUPS, GU, 128, RSW).transpose(0, 1, 3, 2, 4)
    return [
        {"l": np.ascontiguousarray(lp[k]), "rs": np.ascontiguousarray(rb[k])}
        for k in range(N_CORES)
    ]


def _unpack_outputs(outs):
    """outs: 8 arrays [NGROUPS, 2, 2, 32, GU, 64, 4, NBLK] -> [B,G,D,H,W]."""
    # dims: [g, k, i3, (4hi,8il), ui, 64j, 4q, 10blk]; partition held
    # ww = 16k + i with i = 8*i3 + il; stored slot j maps to matmul col
    # n = 16k + j, so d = n - ww = j - i (valid j in [i, i+48))
    O = np.stack([
        np.asarray(o, dtype=np.float32).transpose(0, 4, 1, 2, 3, 5, 6, 7)
        for o in outs])
    # [8core, NG, GU, 2k, 2i3, 32p, 64j, 4q, 10blk] -> unit-major
    O = O.reshape(PAIRS, HGROUPS, 2, 2, 4, 8, 64, 4, NBLK)
    WPAD = 368
    final = np.zeros((PAIRS, D, H, WPAD), dtype=np.float32)
    s0, sd, sh, sw = (np.array(final.strides) // final.itemsize)
    st = np.lib.stride_tricks.as_strided
    it = final.itemsize
    for q in range(4):
        for hi in range(4):
            for k in range(2):
                h0 = 4 * q + hi
                # [80, 6, 2i3, 8il, 64j, 10blk]
                A = O[:, :, k, :, hi, :, :, q, :]
                a = np.array(A.strides) // it
                # V[pair, hg, i3, il, blk, d] = A[.., i3, il, 8*i3+il+d, blk]
                V = st(A, shape=(PAIRS, HGROUPS, 2, 8, NBLK, D),
                       strides=tuple(np.array(
                           [a[0], a[1], a[2] + 8 * a[4], a[3] + a[4],
                            a[5], a[4]]) * it))
                # dest: final[pair, d, 16*hg+h0, 32*blk + 16*k + 8*i3+il + d]
                Dv = st(final[:, :, h0:, 16 * k:],
                        shape=(PAIRS, HGROUPS, 2, 8, NBLK, D),
                        strides=tuple(np.array(
                            [s0, 16 * sh, 8 * sw, sw, MBLK * sw,
                             sd + sw]) * it))
                Dv[...] = V
    return final[:, :, :, :W].reshape(B, G, D, H, W)


def _install_profile_hook():
    """Make trace=True work when the image's antenv lacks axon_hooks."""
    import sys
    import types
    try:
        from antenv.axon_hooks import get_axon_ntff_profile_hook  # noqa: F401
        return
    except ImportError:
        pass
    if "/root/.axon_site" not in sys.path:
        sys.path.insert(0, "/root/.axon_site")
    from trn_agent_boot.trn_boot import _ntff_profile_via_ctypes
    hook = _ntff_profile_via_ctypes("/opt/axon/libaxon_pjrt.so")
    import antenv
    mod = types.ModuleType("antenv.axon_hooks")
    state = {"hook": hook}
    mod.get_axon_ntff_profile_hook = lambda: state["hook"]
    mod.set_axon_ntff_profile_hook = lambda h: state.update(hook=h)
    sys.modules["antenv.axon_hooks"] = mod
    antenv.axon_hooks = mod


def kernel(left_feature, right_feature, max_disp):
    import sys
    if "/opt/trn_rl_repo" not in sys.path:
        sys.path.insert(0, "/opt/trn_rl_repo")
    from concourse import bass_utils
    from concourse.bass_utils import run_bass_kernel_spmd

    left = np.asarray(left_feature, dtype=np.float32)
    right = np.asarray(right_feature, dtype=np.float32)
    assert int(max_disp) == D
    assert left.shape == (B, G * CPG, H, W)

    dt_in_name = os.environ.get("GWC_DT_IN", "bfloat16")
    dt_out_name = os.environ.get("GWC_DT_OUT", "bfloat16")
    if dt_in_name == "bfloat16":
        import ml_dtypes
        dt_np = ml_dtypes.bfloat16
    else:
        dt_np = np.float32
    nc = _get_nc((dt_in_name, dt_out_name))
    in_maps = _pack_inputs(left, right, dt_np)

    trace = bool(os.environ.get("GWC_PROFILE"))
    if trace:
        _install_profile_hook()
        bass_utils.upload_artifacts = lambda tmpdir: str(tmpdir)  # no bucket
    res = run_bass_kernel_spmd(
        nc, in_maps, core_ids=list(range(N_CORES)), trace=trace
    )
    if trace:
        kernel._last_profile = res
        print(f"[kernel] exec_time_ns={res.exec_time_ns} "
              f"mean={res.mean_exec_time_ns}", flush=True)
    outs = [res.results[k]["o"] for k in range(N_CORES)]
    return _unpack_outputs(outs)



# revision 26
# speedup vs baseline: 1.1402x; 1.1402x over previous
"""GwcVolume (group-wise correlation volume) Bass kernel for Trainium2.

Problem: left/right features [2, 320, 96, 312] fp32, GROUP=40, cpg=8,
max_disp=48.  Output cost volume [2, 40, 48, 96, 312]:
    cost[b,g,d,h,w] = mean_c( l[b,g,c,h,w] * r[b,g,c,h,w-d] ),  0 for w<d.

Strategy (8 NeuronCores):
  - Shard the 80 (b,g) pairs across cores, 10 per core.  Each pair is fully
    independent (no collectives).
  - TensorE does all multiply-accumulate work as block-diagonal matmuls:
    for each (bg, h-group of 16), SBUF holds l as [128 = 16h x 8c, W] and a
    host-prebuilt block-diagonal stationary image rs [128, 10*128] where
    the (unit, w'-block blk, h-quad q) stationary is
        rs[32q + 8hi + c, 128 blk + 32 hi + ww] = r[h, c, 32 blk + ww] / 0,
    h = 16 hg + 4 q + hi.  matmul (K=32 rows at strip 32q, M=128, N=79):
        out[(hi,ww), n] = sum_c r[h,c,w'0+ww] * l[h,c,w'0+n]
                        = cost[d=n-ww, h, w=w'0+n]  for 0 <= n-ww < 48.
    The 4 quads run on distinct PE row-strips and distinct PSUM banks,
    so they execute concurrently on the 32x32 sub-array grid.
  - VectorE/ScalarE evacuate PSUM into a w-major SBUF buffer, DMA'd to HBM
    densely.  The host does the final (free) rearrangement: band extraction
    (d = n - ww), zero triangle for w < d, and the layout transpose.

The device never performs the (d,w)-diagonal transpose -- that keeps every
DMA fully contiguous; the host does it with numpy stride tricks.
"""

import os

import numpy as np

# --- geometry (hardcoded for this problem) ---
B, G, CPG, H, W = 2, 40, 8, 96, 312
D = 48                      # max_disp
N_CORES = 8
PAIRS = B * G               # 80 (b,g) pairs
BG_PER_CORE = PAIRS // N_CORES  # 10
HGROUPS = H // 16           # 6 groups of 16 h's
NBLK = 10                   # w'-blocks of 32 (covers w' in [0, 320))
MBLK = 32                   # w' per block
NW = MBLK + D - 1           # 79 moving columns per matmul
WL = 368                    # padded l width (312 + 56; max needed w = 366)
WR = 320                    # padded r width (312 + 8)
UNITS = BG_PER_CORE * HGROUPS   # 60 (bg, hgroup) units per core
RSW = NBLK * 128            # 1280 stationary-image cols per unit
GU = 6                      # units per DMA group (batch DMAs for efficiency)
NGROUPS = UNITS // GU       # 10

_NC_CACHE = {}


def _build_nc(dt_in_name="bfloat16", dt_out_name="bfloat16"):
    from concourse import bacc, mybir, tile
    import concourse.bass as bass  # noqa: F401

    dt_in = getattr(mybir.dt, dt_in_name)
    dt_out = getattr(mybir.dt, dt_out_name)
    f32 = mybir.dt.float32

    nc = bacc.Bacc("TRN2", target_bir_lowering=False, debug=False)
    l_dram = nc.dram_tensor(
        "l", [NGROUPS, 128, GU, WL], dt_in, kind="ExternalInput")
    r_dram = nc.dram_tensor(
        "rs", [NGROUPS, 128, GU, RSW], dt_in, kind="ExternalInput")
    # Band compaction via slot-major ev layout: stationary M-cols are
    # permuted (host-side) so PSUM partition p = 64k + 16hi + i holds
    # ww = 16k + i, making the valid 48-wide band of partitions [0,64)
    # sit inside moving cols [0,64) and of [64,128) inside [16,80).
    # ev stores the matmul col index ("slot") OUTER-most per partition,
    # so each half's 64-slot window is one contiguous run and the
    # out-DMA extracts a 64/48 = 1.33x band (vs 79/48 = 1.65x) with
    # ~5KB descriptors.  Copies and matmuls are unchanged in count.
    NWM = 80                    # matmul cols (covers window [16,80))
    o_dram = nc.dram_tensor(
        "o", [NGROUPS, 2, 64, GU, 64, 4, NBLK], dt_out,
        kind="ExternalOutput")

    HB = NBLK // 2              # 5 blks per psum tile (one bank per quad)
    SEG = 96                    # col stride of a blk inside its bank
    GH = GU // 2                # out-DMA half-group granularity

    with tile.TileContext(nc) as tc:
        with (
            tc.tile_pool(name="lp", bufs=2) as lp,
            tc.tile_pool(name="rp", bufs=2) as rp,
            tc.tile_pool(name="evp", bufs=2) as evp,
            tc.tile_pool(name="psp", bufs=2, space="PSUM") as psp,
        ):
            for g in range(NGROUPS):
                lt = lp.tile([128, GU, WL], dt_in)
                rt = rp.tile([128, GU, RSW], dt_in)
                # split loads in halves: the first half-group's compute can
                # start while the second half streams in
                for u0 in (0, GH):
                    nc.sync.dma_start(
                        lt[:, u0:u0 + GH, :], l_dram[g, :, u0:u0 + GH, :])
                    nc.sync.dma_start(
                        rt[:, u0:u0 + GH, :], r_dram[g, :, u0:u0 + GH, :])
                ev = evp.tile([128, GU, NWM, 4, NBLK], dt_out)
                for ui in range(GU):
                    for h in range(2):
                        # 5 blks share one psum tile: quad q in bank q,
                        # blk s at col 96*s
                        ps = psp.tile([128, 4, 512], f32)
                        for s in range(HB):
                            blk = h * HB + s
                            for q in range(4):
                                nc.tensor.matmul(
                                    out=ps[:, q, SEG * s:SEG * s + NWM],
                                    lhsT=rt[32 * q:32 * q + 32, ui,
                                            128 * blk:128 * blk + 128],
                                    rhs=lt[32 * q:32 * q + 32, ui,
                                           MBLK * blk:MBLK * blk + NWM],
                                    start=True,
                                    stop=True,
                                    tile_position=(32 * q, 0),
                                )
                        # [p, q, s, n] -> [p, n, q, s] slot-major store
                        src = ps[:, :, 0:HB * SEG].rearrange(
                            "p q (s n) -> p n q s", n=SEG)[:, 0:NWM, :, :]
                        dst = ev[:, ui, :, :, h * HB:(h + 1) * HB]
                        if h == 0:
                            nc.vector.tensor_copy(out=dst, in_=src)
                        else:
                            nc.scalar.copy(out=dst, in_=src)
                    # out-DMA per (half-group, partition-half); each reads its
                    # half's contiguous 64-slot band window.  k=0 targets
                    # even SDMA engines (p<64) and k=1 odd (p>=64): issue
                    # them on SEPARATE HWDGE rings (ACT / sync) so both
                    # engine-parity sets drain concurrently from
                    # independent descriptor streams
                    if ui == GH - 1 or ui == GU - 1:
                        u0 = ui + 1 - GH
                        for k in range(2):
                            eng = nc.scalar if k == 0 else nc.sync
                            eng.dma_start(
                                o_dram[g, k, :, u0:u0 + GH],
                                ev[64 * k:64 * k + 64, u0:u0 + GH,
                                   16 * k:16 * k + 64, :, :])
    nc.compile()
    return nc


def _get_nc(key=("bfloat16", "bfloat16")):
    if key not in _NC_CACHE:
        _NC_CACHE[key] = _build_nc(*key)
    return _NC_CACHE[key]


def _pack_inputs(left, right, dt_np):
    """-> per-core in_maps; l pre-scaled by 1/cpg, r as block-diag image."""
    # [B, C, H, W] -> [B, G, cpg, H, W] -> [pair, H, cpg, W]
    l5 = left.reshape(B, G, CPG, H, W).transpose(0, 1, 3, 2, 4).reshape(
        PAIRS, H, CPG, W)
    r5 = right.reshape(B, G, CPG, H, W).transpose(0, 1, 3, 2, 4).reshape(
        PAIRS, H, CPG, W)
    lp = np.zeros((PAIRS, H, CPG, WL), dtype=np.float32)
    lp[..., :W] = l5 * (1.0 / CPG)
    lp = lp.astype(dt_np)
    # l: [pair, H=6*16, cpg, WL] -> per core [NGROUPS, 128, GU, WL]
    lp = lp.reshape(N_CORES, NGROUPS, GU, 128, WL).transpose(0, 1, 3, 2, 4)

    rp = np.zeros((PAIRS, H, CPG, WR), dtype=np.float32)
    rp[..., :W] = r5
    rp = rp.astype(dt_np)
    # block-diagonal stationary image:
    # axes: [pair, hg, q, hi_row, c, blk, hi_col, ww]
    rv = rp.reshape(PAIRS, HGROUPS, 4, 4, CPG, NBLK, MBLK)
    rb = np.zeros((PAIRS, HGROUPS, 4, 4, CPG, NBLK, 4, MBLK), dtype=dt_np)
    for i in range(4):
        rb[:, :, :, i, :, :, i, :] = rv[:, :, :, i, :, :, :]
    # permute M-cols (hi, ww=16k+i) -> (k, hi, i) for band compaction
    rb = rb.reshape(PAIRS, HGROUPS, 4, 4, CPG, NBLK, 4, 2, 16)
    rb = np.ascontiguousarray(rb.transpose(0, 1, 2, 3, 4, 5, 7, 6, 8))
    rb = rb.reshape(N_CORES, NGROUPS, GU, 128, RSW).transpose(0, 1, 3, 2, 4)
    return [
        {"l": np.ascontiguousarray(lp[k]), "rs": np.ascontiguousarray(rb[k])}
        for k in range(N_CORES)
    ]


def _unpack_outputs(outs):
    """outs: 8 arrays [NGROUPS, 2, 64, GU, 64, 4, NBLK] -> [B,G,D,H,W]."""
    # dims: [g, k, (4hi,16i), ui, 64j, 4q, 10blk]; partition held
    # ww = 16k + i, stored slot j maps to matmul col n = 16k + j, so
    # d = n - ww = j - i (valid j in [i, i+48))
    O = np.stack([
        np.asarray(o, dtype=np.float32).transpose(0, 3, 1, 2, 4, 5, 6)
        for o in outs])
    # [8core, NG, GU, 2k, 4hi, 16i, 64j, 4q, 10blk] -> unit-major
    O = O.reshape(PAIRS, HGROUPS, 2, 4, 16, 64, 4, NBLK)
    WPAD = 368
    final = np.zeros((PAIRS, D, H, WPAD), dtype=np.float32)
    s0, sd, sh, sw = (np.array(final.strides) // final.itemsize)
    st = np.lib.stride_tricks.as_strided
    it = final.itemsize
    for q in range(4):
        for hi in range(4):
            for k in range(2):
                h0 = 4 * q + hi
                A = O[:, :, k, hi, :, :, q, :]  # [80, 6, 16i, 64j, 10blk]
                a = np.array(A.strides) // it
                # V[pair, hg, i, blk, d] = A[pair, hg, i, i+d, blk]
                V = st(A, shape=(PAIRS, HGROUPS, 16, NBLK, D),
                       strides=tuple(np.array([a[0], a[1], a[2] + a[3],
                                               a[4], a[3]]) * it))
                # dest: final[pair, d, 16*hg + h0, 32*blk + 16*k + i + d]
                Dv = st(final[:, :, h0:, 16 * k:],
                        shape=(PAIRS, HGROUPS, 16, NBLK, D),
                        strides=tuple(np.array([s0, 16 * sh, sw, MBLK * sw,
                                                sd + sw]) * it))
                Dv[...] = V
    return final[:, :, :, :W].reshape(B, G, D, H, W)


def _install_profile_hook():
    """Make trace=True work when the image's antenv lacks axon_hooks."""
    import sys
    import types
    try:
        from antenv.axon_hooks import get_axon_ntff_profile_hook  # noqa: F401
        return
    except ImportError:
        pass
    if "/root/.axon_site" not in sys.path:
        sys.path.insert(0, "/root/.axon_site")
    from trn_agent_boot.trn_boot import _ntff_profile_via_ctypes
    hook = _ntff_profile_via_ctypes("/opt/axon/libaxon_pjrt.so")
    import antenv
    mod = types.ModuleType("antenv.axon_hooks")
    state = {"hook": hook}
    mod.get_axon_ntff_profile_hook = lambda: state["hook"]
    mod.set_axon_ntff_profile_hook = lambda h: state.update(hook=h)
    sys.modules["antenv.axon_hooks"] = mod
    antenv.axon_hooks = mod


def kernel(left_feature, right_feature, max_disp):
    import sys
    if "/opt/trn_rl_repo" not in sys.path:
        sys.path.insert(0, "/opt/trn_rl_repo")
    from concourse import bass_utils
    from concourse.bass_utils import run_bass_kernel_spmd

    left = np.asarray(left_feature, dtype=np.float32)
    right = np.asarray(right_feature, dtype=np.float32)
    assert int(max_disp) == D
    assert left.shape == (B, G * CPG, H, W)

    dt_in_name = os.environ.get("GWC_DT_IN", "bfloat16")
    dt_out_name = os.environ.get("GWC_DT_OUT", "bfloat16")
    if dt_in_name == "bfloat16":
        import ml_dtypes
        dt_np = ml_dtypes.bfloat16
    else:
        dt_np = np.float32
    nc = _get_nc((dt_in_name, dt_out_name))
    in_maps = _pack_inputs(left, right, dt_np)

    trace = bool(os.environ.get("GWC_PROFILE"))
    if trace:
        _install_profile_hook()
        bass_utils.upload_artifacts = lambda tmpdir: str(tmpdir)  # no bucket
    res = run_bass_kernel_spmd(
        nc, in_maps, core_ids=list(range(N_CORES)), trace=trace
    )
    if trace:
        kernel._last_profile = res
        print(f"[kernel] exec_time_ns={res.exec_time_ns} "
              f"mean={res.mean_exec_time_ns}", flush=True)
    outs = [res.results[k]["o"] for k in range(N_CORES)]
    return _unpack_outputs(outs)



# revision 27
# speedup vs baseline: 1.1961x; 1.0490x over previous
"""GwcVolume (group-wise correlation volume) Bass kernel for Trainium2.

Problem: left/right features [2, 320, 96, 312] fp32, GROUP=40, cpg=8,
max_disp=48.  Output cost volume [2, 40, 48, 96, 312]:
    cost[b,g,d,h,w] = mean_c( l[b,g,c,h,w] * r[b,g,c,h,w-d] ),  0 for w<d.

Strategy (8 NeuronCores):
  - Shard the 80 (b,g) pairs across cores, 10 per core.  Each pair is fully
    independent (no collectives).
  - TensorE does all multiply-accumulate work as block-diagonal matmuls:
    for each (bg, h-group of 16), SBUF holds l as [128 = 16h x 8c, W] and a
    host-prebuilt block-diagonal stationary image rs [128, 10*128] where
    the (unit, w'-block blk, h-quad q) stationary is
        rs[32q + 8hi + c, 128 blk + 32 hi + ww] = r[h, c, 32 blk + ww] / 0,
    h = 16 hg + 4 q + hi.  matmul (K=32 rows at strip 32q, M=128, N=79):
        out[(hi,ww), n] = sum_c r[h,c,w'0+ww] * l[h,c,w'0+n]
                        = cost[d=n-ww, h, w=w'0+n]  for 0 <= n-ww < 48.
    The 4 quads run on distinct PE row-strips and distinct PSUM banks,
    so they execute concurrently on the 32x32 sub-array grid.
  - VectorE/ScalarE evacuate PSUM into a w-major SBUF buffer, DMA'd to HBM
    densely.  The host does the final (free) rearrangement: band extraction
    (d = n - ww), zero triangle for w < d, and the layout transpose.

The device never performs the (d,w)-diagonal transpose -- that keeps every
DMA fully contiguous; the host does it with numpy stride tricks.
"""

import os

import numpy as np

# --- geometry (hardcoded for this problem) ---
B, G, CPG, H, W = 2, 40, 8, 96, 312
D = 48                      # max_disp
N_CORES = 8
PAIRS = B * G               # 80 (b,g) pairs
BG_PER_CORE = PAIRS // N_CORES  # 10
HGROUPS = H // 16           # 6 groups of 16 h's
NBLK = 10                   # w'-blocks of 32 (covers w' in [0, 320))
MBLK = 32                   # w' per block
NW = MBLK + D - 1           # 79 moving columns per matmul
WL = 368                    # padded l width (312 + 56; max needed w = 366)
WR = 320                    # padded r width (312 + 8)
UNITS = BG_PER_CORE * HGROUPS   # 60 (bg, hgroup) units per core
RSW = NBLK * 128            # 1280 stationary-image cols per unit
GU = 6                      # units per DMA group (batch DMAs for efficiency)
NGROUPS = UNITS // GU       # 10

_NC_CACHE = {}


def _build_nc(dt_in_name="bfloat16", dt_out_name="bfloat16"):
    from concourse import bacc, mybir, tile
    import concourse.bass as bass  # noqa: F401

    dt_in = getattr(mybir.dt, dt_in_name)
    dt_out = getattr(mybir.dt, dt_out_name)
    f32 = mybir.dt.float32

    nc = bacc.Bacc("TRN2", target_bir_lowering=False, debug=False)
    l_dram = nc.dram_tensor(
        "l", [NGROUPS, 128, GU, WL], dt_in, kind="ExternalInput")
    r_dram = nc.dram_tensor(
        "rs", [NGROUPS, 128, GU, RSW], dt_in, kind="ExternalInput")
    # Band compaction via slot-major ev layout: stationary M-cols are
    # permuted (host-side) so PSUM partition p = 64k + 16hi + i holds
    # ww = 16k + i, making the valid 48-wide band of partitions [0,64)
    # sit inside moving cols [0,64) and of [64,128) inside [16,80).
    # ev stores the matmul col index ("slot") OUTER-most per partition,
    # so each half's 64-slot window is one contiguous run and the
    # out-DMA extracts a 64/48 = 1.33x band (vs 79/48 = 1.65x) with
    # ~5KB descriptors.  Copies and matmuls are unchanged in count.
    NWM = 80                    # matmul cols (covers window [16,80))
    o_dram = nc.dram_tensor(
        "o", [NGROUPS, 2, 64, GU, 64, 4, NBLK], dt_out,
        kind="ExternalOutput")

    HB = NBLK // 2              # 5 blks per psum tile (one bank per quad)
    SEG = 96                    # col stride of a blk inside its bank
    GH = GU // 2                # out-DMA half-group granularity

    with tile.TileContext(nc) as tc:
        with (
            tc.tile_pool(name="lp", bufs=2) as lp,
            tc.tile_pool(name="rp", bufs=2) as rp,
            tc.tile_pool(name="evp", bufs=3) as evp,
            tc.tile_pool(name="psp", bufs=2, space="PSUM") as psp,
        ):
            for g in range(NGROUPS):
                lt = lp.tile([128, GU, WL], dt_in)
                rt = rp.tile([128, GU, RSW], dt_in)
                # split loads in halves: the first half-group's compute can
                # start while the second half streams in
                for u0 in (0, GH):
                    nc.sync.dma_start(
                        lt[:, u0:u0 + GH, :], l_dram[g, :, u0:u0 + GH, :])
                    nc.sync.dma_start(
                        rt[:, u0:u0 + GH, :], r_dram[g, :, u0:u0 + GH, :])
                ev = evp.tile([128, GU, NWM, 4, NBLK], dt_out)
                for ui in range(GU):
                    for h in range(2):
                        # 5 blks share one psum tile: quad q in bank q,
                        # blk s at col 96*s
                        ps = psp.tile([128, 4, 512], f32)
                        for s in range(HB):
                            blk = h * HB + s
                            for q in range(4):
                                nc.tensor.matmul(
                                    out=ps[:, q, SEG * s:SEG * s + NWM],
                                    lhsT=rt[32 * q:32 * q + 32, ui,
                                            128 * blk:128 * blk + 128],
                                    rhs=lt[32 * q:32 * q + 32, ui,
                                           MBLK * blk:MBLK * blk + NWM],
                                    start=True,
                                    stop=True,
                                    tile_position=(32 * q, 0),
                                )
                        # [p, q, s, n] -> [p, n, q, s] slot-major store
                        src = ps[:, :, 0:HB * SEG].rearrange(
                            "p q (s n) -> p n q s", n=SEG)[:, 0:NWM, :, :]
                        dst = ev[:, ui, :, :, h * HB:(h + 1) * HB]
                        if h == 0:
                            nc.vector.tensor_copy(out=dst, in_=src)
                        else:
                            nc.scalar.copy(out=dst, in_=src)
                    # out-DMA per (half-group, partition-half); each reads its
                    # half's contiguous 64-slot band window.  k=0 targets
                    # even SDMA engines (p<64) and k=1 odd (p>=64): issue
                    # them on SEPARATE HWDGE rings (ACT / sync) so both
                    # engine-parity sets drain concurrently from
                    # independent descriptor streams
                    if ui == GH - 1 or ui == GU - 1:
                        u0 = ui + 1 - GH
                        for k in range(2):
                            eng = nc.scalar if k == 0 else nc.sync
                            eng.dma_start(
                                o_dram[g, k, :, u0:u0 + GH],
                                ev[64 * k:64 * k + 64, u0:u0 + GH,
                                   16 * k:16 * k + 64, :, :])
    nc.compile()
    return nc


def _get_nc(key=("bfloat16", "bfloat16")):
    if key not in _NC_CACHE:
        _NC_CACHE[key] = _build_nc(*key)
    return _NC_CACHE[key]


def _pack_inputs(left, right, dt_np):
    """-> per-core in_maps; l pre-scaled by 1/cpg, r as block-diag image."""
    # [B, C, H, W] -> [B, G, cpg, H, W] -> [pair, H, cpg, W]
    l5 = left.reshape(B, G, CPG, H, W).transpose(0, 1, 3, 2, 4).reshape(
        PAIRS, H, CPG, W)
    r5 = right.reshape(B, G, CPG, H, W).transpose(0, 1, 3, 2, 4).reshape(
        PAIRS, H, CPG, W)
    lp = np.zeros((PAIRS, H, CPG, WL), dtype=np.float32)
    lp[..., :W] = l5 * (1.0 / CPG)
    lp = lp.astype(dt_np)
    # l: [pair, H=6*16, cpg, WL] -> per core [NGROUPS, 128, GU, WL]
    lp = lp.reshape(N_CORES, NGROUPS, GU, 128, WL).transpose(0, 1, 3, 2, 4)

    rp = np.zeros((PAIRS, H, CPG, WR), dtype=np.float32)
    rp[..., :W] = r5
    rp = rp.astype(dt_np)
    # block-diagonal stationary image:
    # axes: [pair, hg, q, hi_row, c, blk, hi_col, ww]
    rv = rp.reshape(PAIRS, HGROUPS, 4, 4, CPG, NBLK, MBLK)
    rb = np.zeros((PAIRS, HGROUPS, 4, 4, CPG, NBLK, 4, MBLK), dtype=dt_np)
    for i in range(4):
        rb[:, :, :, i, :, :, i, :] = rv[:, :, :, i, :, :, :]
    # permute M-cols (hi, ww=16k+i) -> (k, hi, i) for band compaction
    rb = rb.reshape(PAIRS, HGROUPS, 4, 4, CPG, NBLK, 4, 2, 16)
    rb = np.ascontiguousarray(rb.transpose(0, 1, 2, 3, 4, 5, 7, 6, 8))
    rb = rb.reshape(N_CORES, NGROUPS, GU, 128, RSW).transpose(0, 1, 3, 2, 4)
    return [
        {"l": np.ascontiguousarray(lp[k]), "rs": np.ascontiguousarray(rb[k])}
        for k in range(N_CORES)
    ]


def _unpack_outputs(outs):
    """outs: 8 arrays [NGROUPS, 2, 64, GU, 64, 4, NBLK] -> [B,G,D,H,W]."""
    # dims: [g, k, (4hi,16i), ui, 64j, 4q, 10blk]; partition held
    # ww = 16k + i, stored slot j maps to matmul col n = 16k + j, so
    # d = n - ww = j - i (valid j in [i, i+48))
    O = np.stack([
        np.asarray(o, dtype=np.float32).transpose(0, 3, 1, 2, 4, 5, 6)
        for o in outs])
    # [8core, NG, GU, 2k, 4hi, 16i, 64j, 4q, 10blk] -> unit-major
    O = O.reshape(PAIRS, HGROUPS, 2, 4, 16, 64, 4, NBLK)
    WPAD = 368
    final = np.zeros((PAIRS, D, H, WPAD), dtype=np.float32)
    s0, sd, sh, sw = (np.array(final.strides) // final.itemsize)
    st = np.lib.stride_tricks.as_strided
    it = final.itemsize
    for q in range(4):
        for hi in range(4):
            for k in range(2):
                h0 = 4 * q + hi
                A = O[:, :, k, hi, :, :, q, :]  # [80, 6, 16i, 64j, 10blk]
                a = np.array(A.strides) // it
                # V[pair, hg, i, blk, d] = A[pair, hg, i, i+d, blk]
                V = st(A, shape=(PAIRS, HGROUPS, 16, NBLK, D),
                       strides=tuple(np.array([a[0], a[1], a[2] + a[3],
                                               a[4], a[3]]) * it))
                # dest: final[pair, d, 16*hg + h0, 32*blk + 16*k + i + d]
                Dv = st(final[:, :, h0:, 16 * k:],
                        shape=(PAIRS, HGROUPS, 16, NBLK, D),
                        strides=tuple(np.array([s0, 16 * sh, sw, MBLK * sw,
                                                sd + sw]) * it))
                Dv[...] = V
    return final[:, :, :, :W].reshape(B, G, D, H, W)


def _install_profile_hook():
    """Make trace=True work when the image's antenv lacks axon_hooks."""
    import sys
    import types
    try:
        from antenv.axon_hooks import get_axon_ntff_profile_hook  # noqa: F401
        return
    except ImportError:
        pass
    if "/root/.axon_site" not in sys.path:
        sys.path.insert(0, "/root/.axon_site")
    from trn_agent_boot.trn_boot import _ntff_profile_via_ctypes
    hook = _ntff_profile_via_ctypes("/opt/axon/libaxon_pjrt.so")
    import antenv
    mod = types.ModuleType("antenv.axon_hooks")
    state = {"hook": hook}
    mod.get_axon_ntff_profile_hook = lambda: state["hook"]
    mod.set_axon_ntff_profile_hook = lambda h: state.update(hook=h)
    sys.modules["antenv.axon_hooks"] = mod
    antenv.axon_hooks = mod


def kernel(left_feature, right_feature, max_disp):
    import sys
    if "/opt/trn_rl_repo" not in sys.path:
        sys.path.insert(0, "/opt/trn_rl_repo")
    from concourse import bass_utils
    from concourse.bass_utils import run_bass_kernel_spmd

    left = np.asarray(left_feature, dtype=np.float32)
    right = np.asarray(right_feature, dtype=np.float32)
    assert int(max_disp) == D
    assert left.shape == (B, G * CPG, H, W)

    dt_in_name = os.environ.get("GWC_DT_IN", "bfloat16")
    dt_out_name = os.environ.get("GWC_DT_OUT", "bfloat16")
    if dt_in_name == "bfloat16":
        import ml_dtypes
        dt_np = ml_dtypes.bfloat16
    else:
        dt_np = np.float32
    nc = _get_nc((dt_in_name, dt_out_name))
    in_maps = _pack_inputs(left, right, dt_np)

    trace = bool(os.environ.get("GWC_PROFILE"))
    if trace:
        _install_profile_hook()
        bass_utils.upload_artifacts = lambda tmpdir: str(tmpdir)  # no bucket
    res = run_bass_kernel_spmd(
        nc, in_maps, core_ids=list(range(N_CORES)), trace=trace
    )
    if trace:
        kernel._last_profile = res
        print(f"[kernel] exec_time_ns={res.exec_time_ns} "
              f"mean={res.mean_exec_time_ns}", flush=True)
    outs = [res.results[k]["o"] for k in range(N_CORES)]
    return _unpack_outputs(outs)

